# revision 2
# baseline (speedup 1.0000x reference)
"""Trainium2 Bass kernel for CrossFeature: out[b, p(i,j)] = x[b,i]*x[b,j]*dot(v[i],v[j]).

Full shapes: x [8192, 300] f32, v [300, 4] f32 -> out [8192, 44850] f32
(P = 300*299/2 upper-triangular pairs, row-major order).

Strategy (data-parallel over 8 NeuronCores, batch-sharded, no cross-core
communication; per core 1024 rows as [128 part, 8 bh, 300]):
  - host: w[p] = (v @ v.T)[i(p), j(p)] as fp16 (tiny), shard x by batch.
  - per output column chunk [c0, c1) of width CHUNK:
      * PE broadcasts the fp16 w chunk into PSUM fp32 via ones[1,128]^T @ w.
      * pass 1 (t = x_i * x_seg), engine by i-range to balance load:
          i < ACT_END            -> ScalarE activation(Copy, scale) per (i, b)
          ACT_END <= i < DVE_END -> DVE tensor_scalar per (i, b); FD padded to
                                    even so the 2x_2P perf mode engages
          DVE_END <= i < GPS_TT  -> GPSIMD tensor_scalar per (i, b)
          i >= GPS_TT            -> GPSIMD tensor_tensor with stride-0
                                    broadcast of x_i (one instr per i)
      * pass 2 (t *= w): one DVE TT [128, 8, cw_d] vs PSUM w (stride-0 mid
        dim), plus a GPSIMD TT on the last CW_G columns from an SBUF copy
        of w made by ScalarE.
      * out DMA on the sync HWDGE ring; input DMAs ride the scalar ring.
"""

import numpy as np

import concourse.bacc as bacc
import concourse.bass as bass
import concourse.mybir as mybir
from concourse.tile import TileContext
from concourse.bass_utils import run_bass_kernel_spmd

N_CORES = 8
B_FULL = 8192
F_FULL = 300

# tuning knobs
CHUNK = 1536        # output columns per tile/DMA
ACT_END = 48        # i <  this -> ScalarE per (i, b)
DVE_END = 120       # .. < this -> DVE tensor_scalar per (i, b)
GPS_TT = 236        # .. < this -> GPSIMD tensor_scalar per (i, b); rest TT
CW_G = 300          # pass-2 columns per chunk handled by GPSIMD


def bcast_last(ap, n):
    """[..., 1] AP -> [..., n] with stride-0 last dim (free-dim broadcast)."""
    a = [list(d) for d in ap.ap]
    assert a[-1][1] == 1, a
    return bass.AP(ap.tensor, ap.offset, a[:-1] + [[0, n]])


def bcast_mid(ap, n):
    """[p, m] AP -> [p, n, m] with a stride-0 middle dim."""
    a = [list(d) for d in ap.ap]
    return bass.AP(ap.tensor, ap.offset, a[:-1] + [[0, n]] + a[-1:])


def chunk_segments(f, c0, c1):
    """Pair-segments of the triu(f, k=1) row-major layout intersected with
    column window [c0, c1). Yields (i, ps, pe, j0): output cols [ps, pe) hold
    x[:, i] * x[:, j0 : j0 + (pe-ps)]."""
    s = 0
    for i in range(f - 1):
        ln = f - 1 - i
        s0, s1 = s, s + ln
        if s0 >= c1:
            break
        if s1 > c0:
            ps, pe = max(s0, c0), min(s1, c1)
            yield i, ps, pe, i + 1 + (ps - s0)
        s = s1


def build_program(bh=8, f=F_FULL, chunk=CHUNK, act_end=ACT_END,
                  dve_end=DVE_END, gps_tt=GPS_TT, cw_g=CW_G,
                  n_cores=N_CORES):
    """Build + compile the per-core Bass program. Shard shape: [bh*128, f]."""
    p_pairs = f * (f - 1) // 2
    rows = bh * 128
    f32 = mybir.dt.float32
    f16 = mybir.dt.float16

    nc = bacc.Bacc("TRN2", target_bir_lowering=False, debug=False,
                   num_devices=n_cores)
    x_d = nc.dram_tensor("x", [rows, f], f32, kind="ExternalInput")
    w_d = nc.dram_tensor("w", [1, p_pairs], f16, kind="ExternalInput")
    o_d = nc.dram_tensor("out", [rows, p_pairs], f32, kind="ExternalOutput")

    fpad = f + 4          # x free-dim pad so even-FD reads can overrun by 1
    opad = chunk + 8      # ob pad for the same overrun on the last segment

    with TileContext(nc) as tc:
        with (
            tc.tile_pool(name="xp", bufs=1) as xp,
            tc.tile_pool(name="wp", bufs=3) as wp,
            tc.tile_pool(name="wb", bufs=2) as wb,
            tc.tile_pool(name="op", bufs=3) as op,
            tc.tile_pool(name="pp", bufs=2, space=bass.MemorySpace.PSUM) as pp,
        ):
            x_sb = xp.tile([128, bh, fpad], f32)
            nc.vector.memset(x_sb[:, :, f:], 0.0)
            nc.scalar.dma_start(
                out=x_sb[:, :, :f],
                in_=x_d.rearrange("(bh bl) f -> bl bh f", bl=128),
            )
            ones = xp.tile([1, 128], f16)
            nc.vector.memset(ones[:], 1.0)

            out_r = o_d.rearrange("(bh bl) p -> bl bh p", bl=128)

            for c0 in range(0, p_pairs, chunk):
                c1 = min(c0 + chunk, p_pairs)
                cw = c1 - c0
                # pass-2 column split: GPSIMD takes the last cw_gc columns
                cw_gc = cw_g if cw >= 512 else 0
                cw_d = cw - cw_gc

                w_sb = wp.tile([1, chunk], f16, tag="w")
                nc.scalar.dma_start(out=w_sb[:, :cw], in_=w_d[:, c0:c1])
                w_ps = pp.tile([128, chunk], f32, tag="wps")
                for n0 in range(0, cw, 512):
                    n1 = min(n0 + 512, cw)
                    nc.tensor.matmul(
                        w_ps[:, n0:n1], ones[:], w_sb[:, n0:n1],
                        start=True, stop=True,
                    )
                if cw_gc:
                    w_bc = wb.tile([128, cw_g], f32, tag="wbc")
                    nc.scalar.copy(w_bc[:, :cw_gc], w_ps[:, cw_d:cw])

                ob = op.tile([128, bh, opad], f32, tag="ob")
                for i, ps, pe, j0 in chunk_segments(f, c0, c1):
                    ln = pe - ps
                    o0 = ps - c0
                    if i < act_end:
                        for b in range(bh):
                            nc.scalar.activation(
                                ob[:, b, o0:o0 + ln],
                                x_sb[:, b, j0:j0 + ln],
                                mybir.ActivationFunctionType.Copy,
                                scale=x_sb[:, b, i:i + 1],
                            )
                    elif i < dve_end:
                        ln2 = ln + (ln & 1)   # even FD -> DVE 2x_2P mode
                        for b in range(bh):
                            nc.vector.tensor_scalar_mul(
                                ob[:, b, o0:o0 + ln2],
                                x_sb[:, b, j0:j0 + ln2],
                                x_sb[:, b, i:i + 1],
                            )
                    elif i < gps_tt:
                        for b in range(bh):
                            nc.gpsimd.tensor_scalar_mul(
                                ob[:, b, o0:o0 + ln],
                                x_sb[:, b, j0:j0 + ln],
                                x_sb[:, b, i:i + 1],
                            )
                    else:
                        nc.gpsimd.tensor_mul(
                            out=ob[:, :, o0:o0 + ln],
                            in0=x_sb[:, :, j0:j0 + ln],
                            in1=bcast_last(x_sb[:, :, i:i + 1], ln),
                        )
                if cw_gc:
                    nc.gpsimd.tensor_mul(
                        out=ob[:, :, cw_d:cw],
                        in0=ob[:, :, cw_d:cw],
                        in1=bcast_mid(w_bc[:, :cw_gc], bh),
                    )
                nc.vector.tensor_mul(
                    out=ob[:, :, :cw_d],
                    in0=ob[:, :, :cw_d],
                    in1=bcast_mid(w_ps[:, :cw_d], bh),
                )
                nc.sync.dma_start(out=out_r[:, :, c0:c1], in_=ob[:, :, :cw])

    nc.compile()
    return nc


def pair_weights(v):
    """w[p] = dot(v[i(p)], v[j(p)]) in row-major triu order, as [1, P] f16."""
    g = v.astype(np.float64) @ v.astype(np.float64).T
    ii, jj = np.triu_indices(v.shape[0], k=1)
    return np.ascontiguousarray(g[ii, jj][None, :].astype(np.float16))


_prog_cache = {}


def _get_program():
    key = (N_CORES, F_FULL, CHUNK, ACT_END, DVE_END, GPS_TT, CW_G)
    if key not in _prog_cache:
        _prog_cache[key] = build_program()
    return _prog_cache[key]


def run(x, v, trace=False, trace_kwargs=None):
    """Run on all 8 cores; returns (out [8192, P] f32, BassKernelResults)."""
    assert x.shape == (B_FULL, F_FULL), x.shape
    nc = _get_program()
    w = pair_weights(np.asarray(v))
    xs = np.ascontiguousarray(np.asarray(x, dtype=np.float32))
    b_loc = B_FULL // N_CORES
    in_maps = [
        {"x": np.ascontiguousarray(xs[c * b_loc:(c + 1) * b_loc]), "w": w}
        for c in range(N_CORES)
    ]
    res = run_bass_kernel_spmd(
        nc, in_maps, list(range(N_CORES)), trace=trace,
        **(trace_kwargs or {}),
    )
    out = np.concatenate([res.results[c]["out"] for c in range(N_CORES)], axis=0)
    return out, res


def kernel(x, v):
    out, _ = run(x, v)
    return out


# revision 7
# speedup vs baseline: 3.0865x; 3.0865x over previous
"""Trainium2 Bass kernel for CrossFeature: out[b, p(i,j)] = x[b,i]*x[b,j]*dot(v[i],v[j]).

Full shapes: x [8192, 300] f32, v [300, 4] f32 -> out [8192, 44850] f32
(P = 300*299/2 upper-triangular pairs, row-major order).

Strategy (data-parallel over 8 NeuronCores, batch-sharded, no cross-core
communication; per core 1024 rows as [128 part, 8 bh, 300]):
  - host: w[p] = (v @ v.T)[i(p), j(p)] as fp16 (tiny), shard x by batch.
  - per output column chunk [c0, c1) of width CHUNK:
      * PE broadcasts the fp16 w chunk into PSUM fp32 via ones[1,128]^T @ w.
      * pass 1 (t = x_i * x_seg), engine by i-range to balance load:
          i < ACT_END            -> ScalarE activation(Copy, scale) per (i, b)
          ACT_END <= i < DVE_END -> DVE tensor_tensor, stride-0 broadcast of
                                    x_i, one instr per i (all 8 b batched)
          i >= DVE_END           -> same on GPSIMD
      * pass 2 (t *= w): one DVE TT [128, 8, cw_d] vs PSUM w (stride-0 mid
        dim), plus a GPSIMD TT on the last CW_G columns from an SBUF copy
        of w made by ScalarE.
      * out DMA on the sync HWDGE ring; input DMAs ride the scalar ring.
"""

import numpy as np

import concourse.bacc as bacc
import concourse.bass as bass
import concourse.mybir as mybir
from concourse.tile import TileContext
from concourse.bass_utils import run_bass_kernel_spmd

N_CORES = 8
B_FULL = 8192
F_FULL = 300

# tuning knobs
CHUNK = 1536        # output columns per tile/DMA
ACT_END = 76        # i <  this -> ScalarE per (i, b)
DVE_END = 116       # .. < this -> DVE tensor_tensor w/ x_i broadcast, per i
CW_G = 192          # pass-2 columns per chunk handled by GPSIMD


def bcast_last(ap, n):
    """[..., 1] AP -> [..., n] with stride-0 last dim (free-dim broadcast)."""
    a = [list(d) for d in ap.ap]
    assert a[-1][1] == 1, a
    return bass.AP(ap.tensor, ap.offset, a[:-1] + [[0, n]])


def bcast_mid(ap, n):
    """[p, m] AP -> [p, n, m] with a stride-0 middle dim."""
    a = [list(d) for d in ap.ap]
    return bass.AP(ap.tensor, ap.offset, a[:-1] + [[0, n]] + a[-1:])


def chunk_segments(f, c0, c1):
    """Pair-segments of the triu(f, k=1) row-major layout intersected with
    column window [c0, c1). Yields (i, ps, pe, j0): output cols [ps, pe) hold
    x[:, i] * x[:, j0 : j0 + (pe-ps)]."""
    s = 0
    for i in range(f - 1):
        ln = f - 1 - i
        s0, s1 = s, s + ln
        if s0 >= c1:
            break
        if s1 > c0:
            ps, pe = max(s0, c0), min(s1, c1)
            yield i, ps, pe, i + 1 + (ps - s0)
        s = s1


def build_program(bh=8, f=F_FULL, chunk=CHUNK, act_end=ACT_END,
                  dve_end=DVE_END, cw_g=CW_G, n_cores=N_CORES):
    """Build + compile the per-core Bass program. Shard shape: [bh*128, f]."""
    p_pairs = f * (f - 1) // 2
    rows = bh * 128
    f32 = mybir.dt.float32
    f16 = mybir.dt.float16

    nc = bacc.Bacc("TRN2", target_bir_lowering=False, debug=False,
                   num_devices=n_cores)
    x_d = nc.dram_tensor("x", [rows, f], f32, kind="ExternalInput")
    w_d = nc.dram_tensor("w", [1, p_pairs], f16, kind="ExternalInput")
    o_d = nc.dram_tensor("out", [rows, p_pairs], f32, kind="ExternalOutput")

    fpad = f + 4          # x free-dim pad so even-FD reads can overrun by 1
    opad = chunk + 8      # ob pad for the same overrun on the last segment

    with TileContext(nc) as tc:
        with (
            tc.tile_pool(name="xp", bufs=1) as xp,
            tc.tile_pool(name="wp", bufs=3) as wp,
            tc.tile_pool(name="wb", bufs=2) as wb,
            tc.tile_pool(name="op", bufs=3) as op,
            tc.tile_pool(name="pp", bufs=2, space=bass.MemorySpace.PSUM) as pp,
        ):
            x_sb = xp.tile([128, bh, fpad], f32)
            nc.vector.memset(x_sb[:, :, f:], 0.0)
            nc.scalar.dma_start(
                out=x_sb[:, :, :f],
                in_=x_d.rearrange("(bh bl) f -> bl bh f", bl=128),
            )
            ones = xp.tile([1, 128], f16)
            nc.vector.memset(ones[:], 1.0)

            out_r = o_d.rearrange("(bh bl) p -> bl bh p", bl=128)

            for c0 in range(0, p_pairs, chunk):
                c1 = min(c0 + chunk, p_pairs)
                cw = c1 - c0
                # pass-2 column split: GPSIMD takes the last cw_gc columns
                cw_gc = cw_g if cw >= 512 else 0
                cw_d = cw - cw_gc

                w_sb = wp.tile([1, chunk], f16, tag="w")
                nc.scalar.dma_start(out=w_sb[:, :cw], in_=w_d[:, c0:c1])
                w_ps = pp.tile([128, chunk], f32, tag="wps")
                for n0 in range(0, cw, 512):
                    n1 = min(n0 + 512, cw)
                    nc.tensor.matmul(
                        w_ps[:, n0:n1], ones[:], w_sb[:, n0:n1],
                        start=True, stop=True,
                    )
                if cw_gc:
                    w_bc = wb.tile([128, cw_g], f32, tag="wbc")
                    nc.scalar.copy(w_bc[:, :cw_gc], w_ps[:, cw_d:cw])

                ob = op.tile([128, bh, opad], f32, tag="ob")
                for i, ps, pe, j0 in chunk_segments(f, c0, c1):
                    ln = pe - ps
                    o0 = ps - c0
                    if i < act_end:
                        for b in range(bh):
                            nc.scalar.activation(
                                ob[:, b, o0:o0 + ln],
                                x_sb[:, b, j0:j0 + ln],
                                mybir.ActivationFunctionType.Copy,
                                scale=x_sb[:, b, i:i + 1],
                            )
                    else:
                        eng = nc.vector if i < dve_end else nc.gpsimd
                        eng.tensor_mul(
                            out=ob[:, :, o0:o0 + ln],
                            in0=x_sb[:, :, j0:j0 + ln],
                            in1=bcast_last(x_sb[:, :, i:i + 1], ln),
                        )
                if cw_gc:
                    nc.gpsimd.tensor_mul(
                        out=ob[:, :, cw_d:cw],
                        in0=ob[:, :, cw_d:cw],
                        in1=bcast_mid(w_bc[:, :cw_gc], bh),
                    )
                nc.vector.tensor_mul(
                    out=ob[:, :, :cw_d],
                    in0=ob[:, :, :cw_d],
                    in1=bcast_mid(w_ps[:, :cw_d], bh),
                )
                nc.sync.dma_start(out=out_r[:, :, c0:c1], in_=ob[:, :, :cw])

    nc.compile()
    return nc


def pair_weights(v):
    """w[p] = dot(v[i(p)], v[j(p)]) in row-major triu order, as [1, P] f16."""
    g = v.astype(np.float64) @ v.astype(np.float64).T
    ii, jj = np.triu_indices(v.shape[0], k=1)
    return np.ascontiguousarray(g[ii, jj][None, :].astype(np.float16))


_prog_cache = {}


def _get_program():
    key = (N_CORES, F_FULL, CHUNK, ACT_END, DVE_END, CW_G)
    if key not in _prog_cache:
        _prog_cache[key] = build_program()
    return _prog_cache[key]


def run(x, v, trace=False, trace_kwargs=None):
    """Run on all 8 cores; returns (out [8192, P] f32, BassKernelResults)."""
    assert x.shape == (B_FULL, F_FULL), x.shape
    nc = _get_program()
    w = pair_weights(np.asarray(v))
    xs = np.ascontiguousarray(np.asarray(x, dtype=np.float32))
    b_loc = B_FULL // N_CORES
    in_maps = [
        {"x": np.ascontiguousarray(xs[c * b_loc:(c + 1) * b_loc]), "w": w}
        for c in range(N_CORES)
    ]
    res = run_bass_kernel_spmd(
        nc, in_maps, list(range(N_CORES)), trace=trace,
        **(trace_kwargs or {}),
    )
    out = np.concatenate([res.results[c]["out"] for c in range(N_CORES)], axis=0)
    return out, res


def kernel(x, v):
    out, _ = run(x, v)
    return out


# revision 8
# speedup vs baseline: 3.5033x; 1.1350x over previous
"""Trainium2 Bass kernel for CrossFeature: out[b, p(i,j)] = x[b,i]*x[b,j]*dot(v[i],v[j]).

Full shapes: x [8192, 300] f32, v [300, 4] f32 -> out [8192, 44850] f32
(P = 300*299/2 upper-triangular pairs, row-major order).

Strategy (data-parallel over 8 NeuronCores, batch-sharded, no cross-core
communication; per core 1024 rows as [128 part, 8 bh, 300]):
  - host: w[p] = (v @ v.T)[i(p), j(p)] as fp16 (tiny), shard x by batch.
  - output columns processed in CHUNK-wide tiles. Each chunk:
      * PE broadcasts the fp16 w chunk into PSUM fp32 via ones[1,128]^T @ w.
      * pass 1 (t = x_i * x_seg): segments split ACT/DVE/GPSIMD *within the
        chunk* by column budgets so every chunk has the same engine profile
        (ACT capped by segment length: its cost is ~fixed per instruction).
        ACT: activation(Copy, scale=x_i) per (i, b); DVE/GPSIMD:
        tensor_tensor with stride-0 broadcast of x_i, one instr per i.
      * pass 2 (t *= w): one DVE TT [128, 8, cw_d] vs PSUM w (stride-0 mid
        dim) + GPSIMD TT on the last P2G columns from an SBUF w copy (ACT).
      * out DMA on the sync HWDGE ring; input DMAs ride the scalar ring.
  - chunks are emitted in interleaved order (0, N-1, 1, N-2, ...) so that
    ACT-heavy early chunks pair with GPSIMD-heavy late chunks in the
    pipeline, and pass-2/DMA of chunk k-1 is emitted after pass-1 of chunk
    k (software pipelining: avoids per-engine FIFO head-of-line blocking).
"""

import numpy as np

import concourse.bacc as bacc
import concourse.bass as bass
import concourse.mybir as mybir
from concourse.tile import TileContext
from concourse.bass_utils import run_bass_kernel_spmd

N_CORES = 8
B_FULL = 8192
F_FULL = 300

# tuning knobs
CHUNK = 1536     # output columns per tile/DMA
A_MAX = 700      # max pass-1 columns per chunk on ScalarE
CA_PER_LN = 2.9  # ScalarE column budget per unit of segment length
G1_BASE = 320    # baseline pass-1 columns per chunk on GPSIMD
G1_SPILL = 0.33  # fraction of the ACT shortfall that spills to GPSIMD
P2G = 448        # pass-2 columns per chunk on GPSIMD


def bcast_last(ap, n):
    """[..., 1] AP -> [..., n] with stride-0 last dim (free-dim broadcast)."""
    a = [list(d) for d in ap.ap]
    assert a[-1][1] == 1, a
    return bass.AP(ap.tensor, ap.offset, a[:-1] + [[0, n]])


def bcast_mid(ap, n):
    """[p, m] AP -> [p, n, m] with a stride-0 middle dim."""
    a = [list(d) for d in ap.ap]
    return bass.AP(ap.tensor, ap.offset, a[:-1] + [[0, n]] + a[-1:])


def chunk_segments(f, c0, c1):
    """Pair-segments of the triu(f, k=1) row-major layout intersected with
    column window [c0, c1). Yields (i, ps, pe, j0): output cols [ps, pe) hold
    x[:, i] * x[:, j0 : j0 + (pe-ps)]."""
    s = 0
    for i in range(f - 1):
        ln = f - 1 - i
        s0, s1 = s, s + ln
        if s0 >= c1:
            break
        if s1 > c0:
            ps, pe = max(s0, c0), min(s1, c1)
            yield i, ps, pe, i + 1 + (ps - s0)
        s = s1


def plan_chunk(f, c0, c1):
    """Segment list + per-engine pass-1 column budgets for one chunk."""
    segs = list(chunk_segments(f, c0, c1))
    cw = c1 - c0
    ln0 = segs[0][2] - segs[0][1]          # longest segment in this chunk
    cap_a = min(A_MAX, int(CA_PER_LN * ln0))
    g1 = min(cw, int(G1_BASE + G1_SPILL * max(0, A_MAX - cap_a)))
    return segs, cw, cap_a, g1


def build_program(bh=8, f=F_FULL, chunk=CHUNK, n_cores=N_CORES):
    """Build + compile the per-core Bass program. Shard shape: [bh*128, f]."""
    p_pairs = f * (f - 1) // 2
    rows = bh * 128
    f32 = mybir.dt.float32
    f16 = mybir.dt.float16

    nc = bacc.Bacc("TRN2", target_bir_lowering=False, debug=False,
                   num_devices=n_cores)
    x_d = nc.dram_tensor("x", [rows, f], f32, kind="ExternalInput")
    w_d = nc.dram_tensor("w", [1, p_pairs], f16, kind="ExternalInput")
    o_d = nc.dram_tensor("out", [rows, p_pairs], f32, kind="ExternalOutput")

    n_chunks = (p_pairs + chunk - 1) // chunk
    # interleave: ACT-heavy early chunks alternate with GPSIMD-heavy late ones
    order = []
    lo, hi = 0, n_chunks - 1
    while lo <= hi:
        order.append(lo)
        if hi != lo:
            order.append(hi)
        lo, hi = lo + 1, hi - 1

    with TileContext(nc) as tc:
        with (
            tc.tile_pool(name="xp", bufs=1) as xp,
            tc.tile_pool(name="wp", bufs=3) as wp,
            tc.tile_pool(name="wb", bufs=2) as wb,
            tc.tile_pool(name="op", bufs=3) as op,
            tc.tile_pool(name="pp", bufs=2, space=bass.MemorySpace.PSUM) as pp,
        ):
            x_sb = xp.tile([128, bh, f], f32)
            nc.scalar.dma_start(
                out=x_sb[:],
                in_=x_d.rearrange("(bh bl) f -> bl bh f", bl=128),
            )
            ones = xp.tile([1, 128], f16)
            nc.vector.memset(ones[:], 1.0)

            out_r = o_d.rearrange("(bh bl) p -> bl bh p", bl=128)

            def emit_pass2_and_dma(st):
                ob, c0, c1, cw, cw_d, cw_gc, w_ps, w_bc = st
                if cw_gc:
                    nc.gpsimd.tensor_mul(
                        out=ob[:, :, cw_d:cw],
                        in0=ob[:, :, cw_d:cw],
                        in1=bcast_mid(w_bc[:, :cw_gc], bh),
                    )
                nc.vector.tensor_mul(
                    out=ob[:, :, :cw_d],
                    in0=ob[:, :, :cw_d],
                    in1=bcast_mid(w_ps[:, :cw_d], bh),
                )
                nc.sync.dma_start(out=out_r[:, :, c0:c1], in_=ob[:, :, :cw])

            prev = None
            for ci in order:
                c0 = ci * chunk
                c1 = min(c0 + chunk, p_pairs)
                segs, cw, cap_a, g1 = plan_chunk(f, c0, c1)
                cw_gc = P2G if cw >= 1024 else 0
                cw_d = cw - cw_gc

                w_sb = wp.tile([1, chunk], f16, tag="w")
                nc.scalar.dma_start(out=w_sb[:, :cw], in_=w_d[:, c0:c1])
                w_ps = pp.tile([128, chunk], f32, tag="wps")
                for n0 in range(0, cw, 512):
                    n1 = min(n0 + 512, cw)
                    nc.tensor.matmul(
                        w_ps[:, n0:n1], ones[:], w_sb[:, n0:n1],
                        start=True, stop=True,
                    )
                w_bc = None
                if cw_gc:
                    w_bc = wb.tile([128, P2G], f32, tag="wbc")
                    nc.scalar.copy(w_bc[:, :cw_gc], w_ps[:, cw_d:cw])

                ob = op.tile([128, bh, chunk], f32, tag="ob")
                used = 0
                for i, ps, pe, j0 in segs:
                    ln = pe - ps
                    o0 = ps - c0
                    if used < cap_a:
                        for b in range(bh):
                            nc.scalar.activation(
                                ob[:, b, o0:o0 + ln],
                                x_sb[:, b, j0:j0 + ln],
                                mybir.ActivationFunctionType.Copy,
                                scale=x_sb[:, b, i:i + 1],
                            )
                    else:
                        eng = nc.gpsimd if used >= cw - g1 else nc.vector
                        eng.tensor_mul(
                            out=ob[:, :, o0:o0 + ln],
                            in0=x_sb[:, :, j0:j0 + ln],
                            in1=bcast_last(x_sb[:, :, i:i + 1], ln),
                        )
                    used += ln

                if prev is not None:
                    emit_pass2_and_dma(prev)
                prev = (ob, c0, c1, cw, cw_d, cw_gc, w_ps, w_bc)
            emit_pass2_and_dma(prev)

    nc.compile()
    return nc


def pair_weights(v):
    """w[p] = dot(v[i(p)], v[j(p)]) in row-major triu order, as [1, P] f16."""
    g = v.astype(np.float64) @ v.astype(np.float64).T
    ii, jj = np.triu_indices(v.shape[0], k=1)
    return np.ascontiguousarray(g[ii, jj][None, :].astype(np.float16))


_prog_cache = {}


def _get_program():
    key = (N_CORES, F_FULL, CHUNK, A_MAX, CA_PER_LN, G1_BASE, G1_SPILL, P2G)
    if key not in _prog_cache:
        _prog_cache[key] = build_program()
    return _prog_cache[key]


def run(x, v, trace=False, trace_kwargs=None):
    """Run on all 8 cores; returns (out [8192, P] f32, BassKernelResults)."""
    assert x.shape == (B_FULL, F_FULL), x.shape
    nc = _get_program()
    w = pair_weights(np.asarray(v))
    xs = np.ascontiguousarray(np.asarray(x, dtype=np.float32))
    b_loc = B_FULL // N_CORES
    in_maps = [
        {"x": np.ascontiguousarray(xs[c * b_loc:(c + 1) * b_loc]), "w": w}
        for c in range(N_CORES)
    ]
    res = run_bass_kernel_spmd(
        nc, in_maps, list(range(N_CORES)), trace=trace,
        **(trace_kwargs or {}),
    )
    out = np.concatenate([res.results[c]["out"] for c in range(N_CORES)], axis=0)
    return out, res


def kernel(x, v):
    out, _ = run(x, v)
    return out
